# revision 2
# baseline (speedup 1.0000x reference)
"""LightGCN-style bipartite GNN propagation on 8 Trainium2 NeuronCores.

Self-contained: kernel(**inputs) -> (gcn_users, gcn_items).

Strategy (edge-parallel, dest-sharded):
- Users/items padded to per-core slices (6656 users + 11520 items per core).
- Each core owns the edges whose DESTINATION falls in its slice; per
  128-row dest block, source rows are fetched with dma_gather (int16
  window-local indices, 4 SWDGE queues round-robin), the segment-sum is
  computed as one-hot matmuls accumulated in PSUM, and the residual term
  is fused on flush.
- After each round the per-core new table slices are AllGathered into
  full replicated tables for the next round's gathers.
- Output = weighted sum of the 5 per-round tables (computed at the end
  from the locally stored own-slices).
"""
import sys
sys.path.insert(0, "/opt/trn_rl_repo")
import numpy as np

import concourse.bass as bass
import concourse.mybir as mybir
import concourse.tile as tile
import concourse.bacc as bacc
from concourse.bass import ds

# ---------------- problem constants (from spec) ----------------
U_NUM, I_NUM, F, E = 52643, 91599, 64, 2_000_000
NCORES, P = 8, 128
USLICE, ISLICE = 6656, 11520          # per-core padded slice sizes
UBLK, IBLK = USLICE // P, ISLICE // P  # 52, 90
U_PAD, I_PAD = NCORES * USLICE, NCORES * ISLICE  # 53248, 92160
UWIN, IWIN = 26624, 23040              # int16 gather windows
UW, IW = U_PAD // UWIN, I_PAD // IWIN  # 2, 4
GB_U, GB_I = 4, 3                      # blocks per loop group
NGRP_U, NGRP_I = UBLK // GB_U, IBLK // GB_I  # 13, 30
W_LAYERS = (1.0, 0.5, 1.0 / 3.0, 0.25, 1.0)
NROUNDS = 5
NQ = 4                                 # SWDGE queues

f32 = mybir.dt.float32
i16 = mybir.dt.int16

_PIPELINE_UNROLL = 2


# ---------------- host-side edge preprocessing ----------------
def _prep_side(dst, src, val0, val1, nblk, dslice, nwin, win):
    """Pack one pass side (U: dst=users src=items, I: dst=items src=users).

    Returns (idx [NBG,128,8*T] i16, dva0, dva1 [NBG,128,2,T] f32, Tw list).
    Edge slot layout: per (global block, window) cell, edges ranked k:
    gather list position k -> msgs slot (tile toff[w]+k//128, partition
    k%128); idx wrapped [p, s] = list[s*16 + p%16].
    """
    NBG = NCORES * nblk
    dst = dst.astype(np.int64)
    src = src.astype(np.int64)
    core = dst // dslice
    lb = (dst % dslice) // P
    gblk = core * nblk + lb
    dstl = (dst % P).astype(np.float32)
    w = src // win
    srcl = (src - w * win).astype(np.int16)

    key = gblk * nwin + w
    order = np.argsort(key, kind="stable")
    ks = key[order]
    cnt = np.bincount(key, minlength=NBG * nwin)
    starts = np.zeros(NBG * nwin + 1, np.int64)
    np.cumsum(cnt, out=starts[1:])
    rank = np.arange(len(dst), dtype=np.int64) - starts[ks]

    Tw = np.maximum(1, -(-cnt.reshape(NBG, nwin).max(axis=0) // P)).astype(int)
    toff = np.zeros(nwin, np.int64)
    toff[1:] = np.cumsum(Tw)[:-1]
    T = int(Tw.sum())

    ws = w[order]
    t_in_blk = toff[ws] + rank // P
    p_slot = (rank % P).astype(np.int64)
    g_sorted = gblk[order]

    idx16 = np.zeros((NBG, 16, 8 * T), np.int16)
    col = 8 * toff[ws] + rank // 16
    row16 = rank % 16
    idx16[g_sorted, row16, col] = srcl[order]
    idx = np.tile(idx16, (1, 8, 1))  # replicate to 128 partitions

    dva0 = np.zeros((NBG, P, 2, T), np.float32)
    dva1 = np.zeros((NBG, P, 2, T), np.float32)
    dva0[g_sorted, p_slot, 0, t_in_blk] = dstl[order]
    dva0[g_sorted, p_slot, 1, t_in_blk] = val0[order]
    dva1[g_sorted, p_slot, 0, t_in_blk] = dstl[order]
    dva1[g_sorted, p_slot, 1, t_in_blk] = val1[order]
    return idx, dva0, dva1, [int(x) for x in Tw]


def _build_kernel(TU_w, TI_w):
    """Build the Bass program. TU_w / TI_w: per-window tile counts."""
    import contextlib

    TU, TI = sum(TU_w), sum(TI_w)
    toff_u = np.concatenate([[0], np.cumsum(TU_w)[:-1]]).astype(int)
    toff_i = np.concatenate([[0], np.cumsum(TI_w)[:-1]]).astype(int)

    nc = bacc.Bacc("TRN2", target_bir_lowering=False, debug=False,
                   num_devices=NCORES, num_swdge_queues=NQ)

    tbl_u0 = nc.dram_tensor("tbl_u0", [U_PAD, F], f32, kind="ExternalInput")
    tbl_i0 = nc.dram_tensor("tbl_i0", [I_PAD, F], f32, kind="ExternalInput")
    idx_u = nc.dram_tensor("idx_u", [UBLK, P, 8 * TU], i16, kind="ExternalInput")
    idx_i = nc.dram_tensor("idx_i", [IBLK, P, 8 * TI], i16, kind="ExternalInput")
    dva_u0 = nc.dram_tensor("dva_u0", [UBLK, P, 2, TU], f32, kind="ExternalInput")
    dva_u1 = nc.dram_tensor("dva_u1", [UBLK, P, 2, TU], f32, kind="ExternalInput")
    dva_i0 = nc.dram_tensor("dva_i0", [IBLK, P, 2, TI], f32, kind="ExternalInput")
    dva_i1 = nc.dram_tensor("dva_i1", [IBLK, P, 2, TI], f32, kind="ExternalInput")
    ego_u = nc.dram_tensor("ego_u", [USLICE, F], f32, kind="ExternalInput")
    ego_i = nc.dram_tensor("ego_i", [ISLICE, F], f32, kind="ExternalInput")
    d_u = nc.dram_tensor("d_u", [UBLK, P, 1], f32, kind="ExternalInput")
    d_i = nc.dram_tensor("d_i", [IBLK, P, 1], f32, kind="ExternalInput")
    iota_t = nc.dram_tensor("iota", [P, P], f32, kind="ExternalInput")
    out_u = nc.dram_tensor("out_u", [USLICE, F], f32, kind="ExternalOutput")
    out_i = nc.dram_tensor("out_i", [ISLICE, F], f32, kind="ExternalOutput")

    qctr = [0]

    with tile.TileContext(nc) as tc:
        with contextlib.ExitStack() as stk:
            const = stk.enter_context(tc.tile_pool(name="const", bufs=1))
            dram = stk.enter_context(tc.tile_pool(name="dram", bufs=1, space="DRAM"))
            upool = stk.enter_context(tc.tile_pool(name="unew", bufs=3))
            wpool = stk.enter_context(tc.tile_pool(name="wt", bufs=6))
            psum = stk.enter_context(tc.tile_pool(name="ps", bufs=8, space="PSUM"))
            accp = stk.enter_context(tc.tile_pool(name="accp", bufs=3))

            iota_sb = const.tile([P, P], f32)
            nc.sync.dma_start(out=iota_sb[:], in_=iota_t[:, :])

            agu = [dram.tile([USLICE, F], f32, name=f"agu{r}", tag=f"agu{r}")
                   for r in range(NROUNDS)]
            agi = [dram.tile([ISLICE, F], f32, name=f"agi{r}", tag=f"agi{r}")
                   for r in range(NROUNDS)]
            tbu = [None] + [dram.tile([U_PAD, F], f32, name=f"tbu{r}",
                                      tag=f"tbu{r}", addr_space="Shared")
                            for r in range(1, NROUNDS)]
            tbi = [None] + [dram.tile([I_PAD, F], f32, name=f"tbi{r}",
                                      tag=f"tbi{r}", addr_space="Shared")
                            for r in range(1, NROUNDS)]

            def make_pass(r, side):
                if side == "U":
                    nblk, GB, NGRP, T, Tw, toff, nwin, win = (
                        UBLK, GB_U, NGRP_U, TU, TU_w, toff_u, IW, IWIN)
                    idx_t = idx_u
                    dva_t = dva_u0 if r == 0 else dva_u1
                    src_tbl = tbl_i0 if r == 0 else tbi[r]
                    res_t = ego_u if r == 0 else agu[r - 1]
                    d_t = d_u
                    dst_t = agu[r]
                else:
                    nblk, GB, NGRP, T, Tw, toff, nwin, win = (
                        IBLK, GB_I, NGRP_I, TI, TI_w, toff_i, UW, UWIN)
                    idx_t = idx_i
                    dva_t = dva_i0 if r == 0 else dva_i1
                    src_tbl = tbl_u0 if r == 0 else tbu[r]
                    res_t = ego_i if r == 0 else agi[r - 1]
                    d_t = d_i
                    dst_t = agi[r]

                C = 8 * T  # idx columns per block

                def stage_load(pipe, g):
                    idx_sb = pipe.intermediate_tile([P, GB * C], i16, name="idx_sb")
                    dva_sb = pipe.intermediate_tile([P, GB * 2 * T], f32, name="dva_sb")
                    res_sb = pipe.intermediate_tile([P, GB * F], f32, name="res_sb")
                    msgs = pipe.intermediate_tile([P, GB * T * F], f32, name="msgs")
                    if r > 0:
                        d_sb = pipe.intermediate_tile([P, GB], f32, name="d_sb")
                        nc.sync.dma_start(
                            out=d_sb[:].rearrange("p (g one) -> p g one", g=GB),
                            in_=d_t[ds(g * GB, GB), :, :].rearrange("g p one -> p g one"),
                        )
                    else:
                        d_sb = None
                    nc.sync.dma_start(
                        out=idx_sb[:].rearrange("p (g c) -> p g c", g=GB),
                        in_=idx_t[ds(g * GB, GB), :, :].rearrange("g p c -> p g c"),
                    )
                    nc.sync.dma_start(
                        out=dva_sb[:].rearrange("p (g x) -> p g x", g=GB),
                        in_=dva_t[ds(g * GB, GB), :, :, :].rearrange(
                            "g p two t -> p g (two t)"),
                    )
                    nc.sync.dma_start(
                        out=res_sb[:].rearrange("p (g f) -> p g f", g=GB),
                        in_=res_t[ds(g * GB * P, GB * P), :].rearrange(
                            "(g p) f -> p g f", p=P),
                    )
                    for j in range(GB):
                        for w in range(nwin):
                            nidx = Tw[w] * P
                            o0 = (j * T + toff[w]) * F
                            nc.gpsimd.dma_gather(
                                out_ap=msgs[:, o0:o0 + Tw[w] * F].rearrange(
                                    "p (t f) -> p t f", f=F),
                                in_ap=src_tbl[w * win:(w + 1) * win, :],
                                idxs_ap=idx_sb[:, j * C + 8 * toff[w]:
                                               j * C + 8 * toff[w] + 8 * Tw[w]],
                                num_idxs=nidx,
                                num_idxs_reg=nidx,
                                elem_size=F,
                                queue_num=qctr[0] % NQ,
                                single_packet=(nidx <= 1024),
                            )
                            qctr[0] += 1
                    if d_sb is None:
                        return idx_sb, dva_sb, res_sb, msgs
                    return idx_sb, dva_sb, res_sb, msgs, d_sb

                def stage_compute(pipe, g, tiles):
                    if r == 0:
                        idx_sb, dva_sb, res_sb, msgs = tiles
                        d_sb = None
                    else:
                        idx_sb, dva_sb, res_sb, msgs, d_sb = tiles
                    unew = upool.tile([P, GB * F], f32, name="unew", tag="unew")
                    for j in range(GB):
                        ps = psum.tile([P, F], f32, name="ps", tag="ps")
                        for t in range(T):
                            Wt = wpool.tile([P, P], f32, name="Wt", tag="Wt")
                            nc.vector.tensor_scalar(
                                out=Wt[:],
                                in0=iota_sb[:],
                                scalar1=dva_sb[:, j * 2 * T + t:j * 2 * T + t + 1],
                                scalar2=dva_sb[:, j * 2 * T + T + t:
                                               j * 2 * T + T + t + 1],
                                op0=mybir.AluOpType.is_equal,
                                op1=mybir.AluOpType.mult,
                            )
                            nc.tensor.matmul(
                                out=ps[:],
                                lhsT=Wt[:],
                                rhs=msgs[:, (j * T + t) * F:(j * T + t + 1) * F],
                                start=(t == 0),
                                stop=(t == T - 1),
                            )
                        nc.vector.scalar_tensor_tensor(
                            out=unew[:, j * F:(j + 1) * F],
                            in0=res_sb[:, j * F:(j + 1) * F],
                            scalar=1.0 if r == 0 else d_sb[:, j:j + 1],
                            in1=ps[:],
                            op0=mybir.AluOpType.mult,
                            op1=mybir.AluOpType.add,
                        )
                    nc.sync.dma_start(
                        out=dst_t[ds(g * GB * P, GB * P), :].rearrange(
                            "(g p) f -> p g f", p=P),
                        in_=unew[:].rearrange("p (g f) -> p g f", g=GB),
                    )

                tc.For_i_pipelined(
                    stages=[stage_load, stage_compute],
                    start=0, end=NGRP, step=1,
                    unroll=_PIPELINE_UNROLL,
                    staged_num_bufs=_PIPELINE_UNROLL,
                    name=f"r{r}{side}",
                )

            for r in range(NROUNDS):
                make_pass(r, "U")
                make_pass(r, "I")
                if r < NROUNDS - 1:
                    nc.gpsimd.collective_compute(
                        "AllGather", mybir.AluOpType.bypass,
                        replica_groups=[list(range(NCORES))],
                        ins=[agu[r][:, :]], outs=[tbu[r + 1][:, :]],
                    )
                    nc.gpsimd.collective_compute(
                        "AllGather", mybir.AluOpType.bypass,
                        replica_groups=[list(range(NCORES))],
                        ins=[agi[r][:, :]], outs=[tbi[r + 1][:, :]],
                    )

            # final: out = sum_r w_r * ag[r], per 128-row block
            for side in ("U", "I"):
                nblk = UBLK if side == "U" else IBLK
                ags = agu if side == "U" else agi
                out_t = out_u if side == "U" else out_i
                for b in range(nblk):
                    rows = slice(b * P, (b + 1) * P)
                    tiles = []
                    for r in range(NROUNDS):
                        tr = accp.tile([P, F], f32, name=f"acc{r}", tag=f"acc{r}")
                        nc.sync.dma_start(out=tr[:], in_=ags[r][rows, :])
                        tiles.append(tr)
                    acc = accp.tile([P, F], f32, name="accv", tag="accv")
                    nc.vector.scalar_tensor_tensor(
                        out=acc[:], in0=tiles[1][:], scalar=W_LAYERS[1],
                        in1=tiles[0][:],
                        op0=mybir.AluOpType.mult, op1=mybir.AluOpType.add)
                    for r in range(2, NROUNDS):
                        nc.vector.scalar_tensor_tensor(
                            out=acc[:], in0=tiles[r][:], scalar=W_LAYERS[r],
                            in1=acc[:],
                            op0=mybir.AluOpType.mult, op1=mybir.AluOpType.add)
                    nc.sync.dma_start(out=out_t[rows, :], in_=acc[:])

    nc.compile()
    return nc


# ---------------- runner (PJRT via axon) ----------------
class _Runner:
    def __init__(self, nc, n_cores):
        import jax
        from jax.sharding import Mesh, PartitionSpec
        from jax.experimental.shard_map import shard_map
        from concourse import bass2jax
        from concourse.bass2jax import _bass_exec_p, install_neuronx_cc_hook

        install_neuronx_cc_hook()
        self.jax = jax
        self.n_cores = n_cores
        in_names, out_names, out_avals, zero_outs = [], [], [], []
        for alloc in nc.m.functions[0].allocations:
            if not isinstance(alloc, mybir.MemoryLocationSet):
                continue
            name = alloc.memorylocations[0].name
            if alloc.kind == "ExternalInput":
                in_names.append(name)
            elif alloc.kind == "ExternalOutput":
                out_names.append(name)
                shape = tuple(alloc.tensor_shape)
                dtype = mybir.dt.np(alloc.dtype)
                out_avals.append(jax.core.ShapedArray(shape, dtype))
                zero_outs.append(np.zeros(shape, dtype))
        partition_name = nc.partition_id_tensor.name if nc.partition_id_tensor else None
        if partition_name is not None:
            in_names = [n for n in in_names if n != partition_name]
        self.in_names, self.out_names = in_names, out_names
        self.out_avals, self.zero_outs = out_avals, zero_outs
        n_params = len(in_names)
        all_names = tuple(in_names + out_names)
        if partition_name is not None:
            all_names = all_names + (partition_name,)

        def _body(*args):
            operands = list(args)
            if partition_name is not None:
                operands.append(bass2jax.partition_id_tensor())
            outs = _bass_exec_p.bind(
                *operands,
                out_avals=tuple(out_avals),
                in_names=all_names,
                out_names=tuple(out_names),
                lowering_input_output_aliases=(),
                sim_require_finite=True,
                sim_require_nnan=True,
                nc=nc,
            )
            return tuple(outs)

        devices = jax.devices()[:n_cores]
        self.mesh = Mesh(np.asarray(devices), ("core",))
        in_specs = (PartitionSpec("core"),) * (n_params + len(out_names))
        out_specs = (PartitionSpec("core"),) * len(out_names)
        self.fn = jax.jit(
            shard_map(_body, mesh=self.mesh, in_specs=in_specs,
                      out_specs=out_specs, check_rep=False),
            keep_unused=True,
        )

    def put_inputs(self, in_maps):
        from jax.sharding import PartitionSpec
        sharding = self.jax.sharding.NamedSharding(self.mesh, PartitionSpec("core"))
        args = []
        for name in self.in_names:
            concat = np.concatenate(
                [np.asarray(in_maps[c][name]) for c in range(self.n_cores)], axis=0)
            args.append(self.jax.device_put(concat, sharding))
        for z in self.zero_outs:
            concat = np.zeros((self.n_cores * z.shape[0], *z.shape[1:]), z.dtype)
            args.append(self.jax.device_put(concat, sharding))
        return args

    def run(self, args):
        outs = self.fn(*args)
        self.jax.block_until_ready(outs)
        return outs

    def results(self, outs):
        res = []
        for c in range(self.n_cores):
            d = {}
            for i, name in enumerate(self.out_names):
                full = np.asarray(outs[i])
                d[name] = full.reshape(self.n_cores, *self.out_avals[i].shape)[c]
            res.append(d)
        return res


_CACHE = {}


def _prepare(embed_user, embed_item, d_i, d_j, ui_u, ui_i,
             ui_vals, iu_vals, adj_vals):
    """Host preprocessing -> (in_maps, kernel-build T params)."""
    idxU, dvaU0, dvaU1, TU_w = _prep_side(
        ui_u, ui_i, adj_vals[:E], ui_vals, UBLK, USLICE, IW, IWIN)
    idxI, dvaI0, dvaI1, TI_w = _prep_side(
        ui_i, ui_u, adj_vals[E:], iu_vals, IBLK, ISLICE, UW, UWIN)

    tbl_u0 = np.zeros((U_PAD, F), np.float32)
    tbl_u0[:U_NUM] = embed_user
    tbl_i0 = np.zeros((I_PAD, F), np.float32)
    tbl_i0[:I_NUM] = embed_item
    du_pad = np.zeros(U_PAD, np.float32)
    du_pad[:U_NUM] = d_i
    dj_pad = np.zeros(I_PAD, np.float32)
    dj_pad[:I_NUM] = d_j
    iota = np.broadcast_to(np.arange(P, dtype=np.float32), (P, P)).copy()

    in_maps = []
    for c in range(NCORES):
        us, ie = c * USLICE, (c + 1) * USLICE
        is_, ii = c * ISLICE, (c + 1) * ISLICE
        in_maps.append({
            "tbl_u0": tbl_u0, "tbl_i0": tbl_i0,
            "idx_u": idxU[c * UBLK:(c + 1) * UBLK],
            "idx_i": idxI[c * IBLK:(c + 1) * IBLK],
            "dva_u0": dvaU0[c * UBLK:(c + 1) * UBLK],
            "dva_u1": dvaU1[c * UBLK:(c + 1) * UBLK],
            "dva_i0": dvaI0[c * IBLK:(c + 1) * IBLK],
            "dva_i1": dvaI1[c * IBLK:(c + 1) * IBLK],
            "ego_u": tbl_u0[us:ie],
            "ego_i": tbl_i0[is_:ii],
            "d_u": du_pad[us:ie].reshape(UBLK, P, 1),
            "d_i": dj_pad[is_:ii].reshape(IBLK, P, 1),
            "iota": iota,
        })
    return in_maps, tuple(TU_w), tuple(TI_w)


def get_runner_and_args(**inputs):
    """Build (cached) kernel + runner and device args for these inputs."""
    inputs = {k: np.asarray(v) for k, v in inputs.items()}
    in_maps, TU_w, TI_w = _prepare(**inputs)
    key = (TU_w, TI_w)
    if key not in _CACHE:
        nc = _build_kernel(list(TU_w), list(TI_w))
        _CACHE[key] = _Runner(nc, NCORES)
    runner = _CACHE[key]
    args = runner.put_inputs(in_maps)
    return runner, args


def kernel(**inputs):
    runner, args = get_runner_and_args(**inputs)
    outs = runner.run(args)
    res = runner.results(outs)
    gcn_users = np.concatenate([res[c]["out_u"] for c in range(NCORES)], axis=0)[:U_NUM]
    gcn_items = np.concatenate([res[c]["out_i"] for c in range(NCORES)], axis=0)[:I_NUM]
    return gcn_users, gcn_items
